# revision 11
# baseline (speedup 1.0000x reference)
"""Trainium2 Bass kernel for EnhancedGNNEncoder (8-core SPMD).

Strategy: shard E=200000 edges across 8 cores (25000 each, padded to 25088 =
49 chunks x 512). Node arrays replicated. Per-edge pipeline per core:
  - dma_gather per-edge node records (x, attr*cw, pooled-col vec, onehot(cs-1))
  - radial quadratic form attr^T D attr on DVE via broadcast-AP tensor ops
  - edge MLP / attention gate / coord MLP on PE in feature-major layout
  - RollerPooling via stacked pool matrices + onehot masking on PE
  - dma_scatter_add partial segment sums -> ReduceScatter -> node phase on
    2500-node shards; host concatenates shard outputs.
"""

import os
import sys

sys.path.insert(0, "/opt/trn_rl_repo")

import numpy as np

N, E, C, D, A, F, H, RNF = 20000, 200000, 14, 3, 16, 128, 128, 128
NCORES = 8
EPC = E // NCORES           # 25000 edges per core
CH = 512                    # chunk of edges per gather/scatter call
NCHUNK = 49                 # ceil(25000/512) -> 25088 padded
EPAD = NCHUNK * CH          # 25088
NPAD = 20480                # scatter-dst rows (N padded to 8*2560)
TRASH = 20400               # row for masked duplicate scatters
NSH = NPAD // NCORES        # 2560 rows per core shard (core 7: 2080 real)
NT = 20                     # node tiles of 128 (20*128 = 2560)
REC = 320                   # packed node record floats (283 used, padded)
LN_EPS = 1e-5


def _pool_mats():
    ones = np.ones((C, C), np.float32)
    mats = []
    for i in range(C):
        ws = C - i
        mats.append((np.triu(ones) - np.triu(ones, k=ws)) / ws)
    return np.stack(mats)  # [C,C,C]  mats[k][r,j]


def _host_prep(inp):
    """Build packed arrays, idx tables, weights. All numpy."""
    h = np.asarray(inp["h"], np.float32)
    x = np.asarray(inp["x"], np.float32)
    attr = np.asarray(inp["channel_attr"], np.float32)
    cw = np.asarray(inp["channel_weights"], np.float32)
    row = np.asarray(inp["row"], np.int32)
    col = np.asarray(inp["col"], np.int32)

    mask = (cw != 0).astype(np.float32)
    cs = mask.sum(1).astype(np.int32)  # [N] channel_sum (>=1 guaranteed)

    rec = np.zeros((N, REC), np.float32)
    rec[:, 0:42] = x.reshape(N, 42)
    rec[:, 42:266] = (attr * cw[:, :, None]).reshape(N, 224)
    rec[:, 266:269] = (x * mask[:, :, None]).sum(1) / cs[:, None]
    rec[np.arange(N), 269 + cs - 1] = 1.0  # onehot(cs-1) at [269:283]

    import ml_dtypes
    h_bf = h.astype(ml_dtypes.bfloat16)

    # per-core idx tables, wrapped [16, 32] per chunk
    def wrap(idx_padded):  # [EPAD] int -> [128, NCHUNK*32] int16 (8x replicated)
        a = idx_padded.reshape(NCHUNK, 32, 16)
        w16 = np.ascontiguousarray(a.transpose(2, 0, 1).reshape(16, -1)).astype(np.int16)
        return np.ascontiguousarray(np.tile(w16, (8, 1)))

    def dedup(idx_pad):
        """Per 128-sub-tile: first-occurrence mask; dups -> TRASH; pads -1."""
        a = idx_pad.reshape(-1, 128)
        sidx = a.copy()
        m = np.zeros(a.shape, np.float32)
        for r_ in range(a.shape[0]):
            rowv = a[r_]
            _, first = np.unique(rowv, return_index=True)
            mm = np.zeros(128, bool); mm[first] = True
            pad = rowv < 0
            m[r_] = mm & ~pad
            sidx[r_][~mm] = TRASH
            sidx[r_][pad] = -1
        return sidx.reshape(-1), m.reshape(-1)

    def subfmt(v):  # [EPAD] -> [128, NCHUNK*4] (token i -> [i%128, i//128])
        return np.ascontiguousarray(v.reshape(-1, 128).T.astype(np.float32))

    tabs = []
    for k in range(NCORES):
        r = row[k * EPC:(k + 1) * EPC]
        c = col[k * EPC:(k + 1) * EPC]
        rg = np.concatenate([r, np.zeros(EPAD - EPC, np.int32)])
        cg = np.concatenate([c, np.zeros(EPAD - EPC, np.int32)])
        rp = np.concatenate([r, -np.ones(EPAD - EPC, np.int32)])
        cp = np.concatenate([c, -np.ones(EPAD - EPC, np.int32)])
        rs, mr = dedup(rp)
        cS, mc = dedup(cp)
        tabs.append((wrap(rg), wrap(cg), wrap(rs), wrap(cS),
                     subfmt(rp), subfmt(cp), subfmt(mr), subfmt(mc)))

    cnt_r = np.bincount(row, minlength=N).astype(np.float32)
    cnt_c = np.bincount(col, minlength=N).astype(np.float32)
    icr = (1.0 / np.maximum(cnt_r, 1.0)).reshape(N, 1)
    icc = (1.0 / np.maximum(cnt_c, 1.0)).reshape(N, 1)

    e_w1 = np.asarray(inp["e_w1"], np.float32)
    rad_w = np.asarray(inp["rad_w"], np.float32)
    mats = _pool_mats().astype(np.float32)
    pstk = np.zeros((14, 196), np.float32)
    repm = np.zeros((14, 196), np.float32)
    selm = np.zeros((196, 14), np.float32)
    for k in range(14):
        for r_ in range(14):
            pstk[:, k * 14 + r_] = mats[k][r_, :]
            repm[k, k * 14 + r_] = 1.0
            selm[k * 14 + r_, r_] = 1.0
    wc = rad_w @ e_w1[256:384, :]  # [256,128]

    W = dict(
        whr=np.ascontiguousarray(e_w1[0:128]),
        whc=np.ascontiguousarray(e_w1[128:256]),
        wc0=np.ascontiguousarray(wc[0:128]),
        wc1=np.ascontiguousarray(wc[128:256]),
        ew2=np.asarray(inp["e_w2"], np.float32),
        attw=np.asarray(inp["att_w"], np.float32),
        ones1=np.ones((1, 128), np.float32),
        cw1=np.asarray(inp["c_w1"], np.float32),
        cw2=np.asarray(inp["c_w2"], np.float32),
        pstk=pstk, repm=repm,
        selm0=np.ascontiguousarray(selm[0:98]), selm1=np.ascontiguousarray(selm[98:196]),
        nw1a=np.ascontiguousarray(np.asarray(inp["n_w1"], np.float32)[0:128]),
        nw1b=np.ascontiguousarray(np.asarray(inp["n_w1"], np.float32)[128:256]),
        nw2=np.asarray(inp["n_w2"], np.float32),
    )
    def padrows(a, fill=0.0):
        out = np.full((NPAD,) + a.shape[1:], fill, np.float32)
        out[:N] = a
        return out
    h_p = padrows(h); x_p = padrows(x.reshape(N, 42))
    icc_p = padrows(icc, 1.0); icr_p = padrows(icr, 1.0)
    shards = []
    for k in range(NCORES):
        sl = slice(k * NSH, (k + 1) * NSH)
        shards.append(dict(
            h_sh=np.ascontiguousarray(h_p[sl]),
            x_sh=np.ascontiguousarray(x_p[sl]),
            icc=np.ascontiguousarray(icc_p[sl]),
            icr=np.ascontiguousarray(icr_p[sl]),
        ))
    return rec, h_bf, tabs, W, shards


# ---------------------------------------------------------------------------

def _build_bass():
    import concourse.bass as bass
    import concourse.bacc as bacc
    import concourse.mybir as mybir
    import concourse.tile as tile
    from concourse.masks import make_identity

    f32 = mybir.dt.float32
    bf16 = mybir.dt.bfloat16
    i16 = mybir.dt.int16
    Alu = mybir.AluOpType
    Act = mybir.ActivationFunctionType

    nc = bacc.Bacc(None)

    # ---- dram io ----
    rec_d = nc.dram_tensor("rec", [N, REC], f32, kind="ExternalInput")
    hbf_d = nc.dram_tensor("h_bf", [N, F], bf16, kind="ExternalInput")
    tabs_d = {nm: nc.dram_tensor(nm, [128, NCHUNK * 32], i16, kind="ExternalInput")
              for nm in ("row_g", "col_g", "row_s", "col_s")}
    sub_d = {nm: nc.dram_tensor(nm, [128, NCHUNK * 4], f32, kind="ExternalInput")
             for nm in ("rowf", "colf", "isfr", "isfc")}
    wshapes = dict(whr=[128, 128], whc=[128, 128], wc0=[128, 128], wc1=[128, 128],
                   ew2=[128, 128], attw=[128, 1], ones1=[1, 128], cw1=[128, 128],
                   cw2=[128, 14], pstk=[14, 196], repm=[14, 196], selm0=[98, 14], selm1=[98, 14],
                   nw1a=[128, 128], nw1b=[128, 128], nw2=[128, 128])
    w_d = {nm: nc.dram_tensor(nm, sh, f32, kind="ExternalInput")
           for nm, sh in wshapes.items()}
    hsh_d = nc.dram_tensor("h_sh", [NSH, F], f32, kind="ExternalInput")
    xsh_d = nc.dram_tensor("x_sh", [NSH, 42], f32, kind="ExternalInput")
    icc_d = nc.dram_tensor("icc", [NSH, 1], f32, kind="ExternalInput")
    icr_d = nc.dram_tensor("icr", [NSH, 1], f32, kind="ExternalInput")

    hout_d = nc.dram_tensor("h_out", [NSH, F], f32, kind="ExternalOutput")
    xout_d = nc.dram_tensor("x_out", [NSH, 42], f32, kind="ExternalOutput")

    agg_d = nc.dram_tensor("agg_sum", [NPAD, F], f32)
    xsum_d = nc.dram_tensor("x_sum", [NPAD, 64], f32)
    aggrs_d = nc.dram_tensor("agg_rs", [NSH, F], f32)
    xsrs_d = nc.dram_tensor("x_rs", [NSH, 64], f32)

    groups = [list(range(NCORES))]

    def apx(base, dims):
        """Rewrap the free dims of a (sliced) AP with explicit [step,count]."""
        return bass.AP(base.tensor, base.offset,
                       [list(base.ap[0])] + [[int(s), int(c)] for s, c in dims])

    with tile.TileContext(nc) as tc:
        with (
            tc.tile_pool(name="wp", bufs=1) as wp,
            tc.tile_pool(name="ep", bufs=2) as ep,
            tc.tile_pool(name="sp", bufs=1) as sp,
            tc.tile_pool(name="pp", bufs=6, space="PSUM") as pp,
        ):
            # ---- load constants ----
            w = {}
            for nm, sh in wshapes.items():
                t = wp.tile(sh, f32, tag=f"w_{nm}")
                nc.sync.dma_start(out=t[:], in_=w_d[nm][:])
                w[nm] = t
            tab = {}
            for nm in tabs_d:
                t = wp.tile([128, NCHUNK * 32], i16, tag=f"t_{nm}")
                nc.sync.dma_start(out=t[:], in_=tabs_d[nm][:])
                tab[nm] = t
            sub = {}
            for nm in sub_d:
                t = wp.tile([128, NCHUNK * 4], f32, tag=f"s_{nm}")
                nc.sync.dma_start(out=t[:], in_=sub_d[nm][:])
                sub[nm] = t
            ident = wp.tile([128, 128], f32, tag="ident")
            make_identity(nc, ident[:])

            # ---- zero scatter accumulators ----
            zt = wp.tile([128, 1536], f32, tag="zeros")
            nc.gpsimd.memset(zt[:], 0.0)

            def zero_dram(dt_, total):
                CHZ = 128 * 1536
                off = 0
                while off < total:
                    n = min(CHZ, total - off)
                    pc = n // 128
                    dst = bass.AP(dt_, off, [[pc, 128], [1, pc]])
                    nc.sync.dma_start(out=dst, in_=zt[:, :pc])
                    off += n
            zero_dram(agg_d, NPAD * F)
            zero_dram(xsum_d, NPAD * 64)

            # =========================== edge phase ===========================
            for cki in range(NCHUNK):
                csl = slice(cki * 32, (cki + 1) * 32)
                nvalid = CH if cki < NCHUNK - 1 else (EPC - (NCHUNK - 1) * CH)

                rec_r = ep.tile([128, 4, REC], f32, tag="rec_r")
                rec_c = ep.tile([128, 4, REC], f32, tag="rec_c")
                nc.gpsimd.dma_gather(rec_r[:], rec_d[:], tab["row_g"][:, csl],
                                     CH, CH, REC)
                nc.gpsimd.dma_gather(rec_c[:], rec_d[:], tab["col_g"][:, csl],
                                     CH, CH, REC)
                hrT = ep.tile([128, 1, CH], bf16, tag="hrT")
                hcT = ep.tile([128, 1, CH], bf16, tag="hcT")
                nc.gpsimd.dma_gather(hrT[:], hbf_d[:], tab["row_g"][:, csl],
                                     CH, CH, F, transpose=True)
                nc.gpsimd.dma_gather(hcT[:], hbf_d[:], tab["col_g"][:, csl],
                                     CH, CH, F, transpose=True)
                hrA = ep.tile([128, CH], f32, tag="hrA")
                hcA = ep.tile([128, CH], f32, tag="hcA")
                nc.scalar.activation(hrA[:], hrT[:, 0, :], Act.Copy)
                nc.scalar.activation(hcA[:], hcT[:, 0, :], Act.Copy)

                RA0 = ep.tile([128, CH], f32, tag="RA0")
                RA1 = ep.tile([128, CH], f32, tag="RA1")
                RAx = ep.tile([128, CH], f32, tag="RAx")
                xsrc = ep.tile([128, 4, 64], f32, tag="xsrc")
                esrc = ep.tile([128, 4, F], f32, tag="esrc")
                nc.gpsimd.memset(xsrc[:], 0.0)

                # ---- radial per 128-edge sub-tile ----
                for s in range(4):
                    rr = rec_r[:, s, :]
                    rc = rec_c[:, s, :]
                    diff = sp.tile([128, 588], f32, tag="diff")
                    nc.vector.tensor_tensor(
                        out=apx(diff[:], [(42, 14), (3, 14), (1, 3)]),
                        in0=apx(rr[:, 0:42], [(3, 14), (0, 14), (1, 3)]),
                        in1=apx(rc[:, 0:42], [(0, 14), (3, 14), (1, 3)]),
                        op=Alu.subtract)
                    sqs = sp.tile([128, 588], f32, tag="sqs")
                    nc.scalar.activation(sqs[:], diff[:], Act.Square)
                    s2 = sp.tile([128, 196], f32, tag="s2")
                    nc.vector.tensor_reduce(
                        out=s2[:], in_=apx(sqs[:], [(3, 196), (1, 3)]),
                        axis=mybir.AxisListType.X, op=Alu.add)
                    dmat = sp.tile([128, 196], f32, tag="dmat")
                    nc.scalar.activation(dmat[:], s2[:], Act.Sqrt)

                    p1 = sp.tile([128, 3136], f32, tag="p1")
                    nc.vector.tensor_tensor(
                        out=apx(p1[:], [(224, 14), (14, 16), (1, 14)]),
                        in0=apx(dmat[:], [(14, 14), (0, 16), (1, 14)]),
                        in1=apx(rc[:, 42:266], [(0, 14), (1, 16), (16, 14)]),
                        op=Alu.mult)
                    Wt = sp.tile([128, 224], f32, tag="Wt")
                    nc.vector.tensor_reduce(
                        out=Wt[:], in_=apx(p1[:], [(14, 224), (1, 14)]),
                        axis=mybir.AxisListType.X, op=Alu.add)
                    p2 = sp.tile([128, 3584], f32, tag="p2")
                    nc.vector.tensor_tensor(
                        out=apx(p2[:], [(224, 16), (14, 16), (1, 14)]),
                        in0=apx(rr[:, 42:266], [(1, 16), (0, 16), (16, 14)]),
                        in1=apx(Wt[:], [(0, 16), (1, 16), (16, 14)]),
                        op=Alu.mult)
                    Rr = sp.tile([128, 256], f32, tag="Rr")
                    nc.vector.tensor_reduce(
                        out=Rr[:], in_=apx(p2[:], [(14, 256), (1, 14)]),
                        axis=mybir.AxisListType.X, op=Alu.add)

                    rn2 = sp.tile([128, 1], f32, tag="rn2")
                    nc.scalar.activation(p2[:, 0:256], Rr[:], Act.Square,
                                         accum_out=rn2[:])
                    rn = sp.tile([128, 1], f32, tag="rn")
                    nc.scalar.activation(rn[:], rn2[:], Act.Sqrt)
                    nc.vector.tensor_scalar_add(out=rn[:], in0=rn[:], scalar1=1.0)
                    inv = sp.tile([128, 1], f32, tag="inv")
                    nc.vector.reciprocal(inv[:], rn[:])

                    aug = sp.tile([128, 384], f32, tag="aug")
                    nc.scalar.activation(aug[:, 0:256], Rr[:], Act.Copy,
                                         scale=inv[:])
                    nc.vector.tensor_copy(aug[:, 256:270], rr[:, 269:283])

                    ssl = slice(s * 128, (s + 1) * 128)
                    for b3, dst in enumerate((RA0, RA1, RAx)):
                        tp = pp.tile([128, CH], f32, tag="ps")
                        nc.tensor.transpose(
                            out=tp[:, 0:128],
                            in_=aug[:, b3 * 128:(b3 + 1) * 128],
                            identity=ident[:])
                        nc.scalar.activation(dst[:, ssl], tp[:, 0:128], Act.Copy)

                # ---- edge MLP (feature-major, PE) ----
                ef1p = pp.tile([128, CH], f32, tag="ps")
                nc.tensor.matmul(out=ef1p[:], lhsT=w["whr"][:], rhs=hrA[:],
                                 start=True, stop=False)
                nc.tensor.matmul(out=ef1p[:], lhsT=w["whc"][:], rhs=hcA[:],
                                 start=False, stop=False)
                nc.tensor.matmul(out=ef1p[:], lhsT=w["wc0"][:], rhs=RA0[:],
                                 start=False, stop=False)
                nc.tensor.matmul(out=ef1p[:], lhsT=w["wc1"][:], rhs=RA1[:],
                                 start=False, stop=True)
                ef1 = ep.tile([128, CH], f32, tag="ef1")
                nc.scalar.activation(ef1[:], ef1p[:], Act.Silu)

                ef2p = pp.tile([128, CH], f32, tag="ps")
                nc.tensor.matmul(out=ef2p[:], lhsT=w["ew2"][:], rhs=ef1[:],
                                 start=True, stop=True)
                ef2 = ep.tile([128, CH], f32, tag="ef2")
                nc.scalar.activation(ef2[:], ef2p[:], Act.Silu)

                gp = pp.tile([128, CH], f32, tag="ps")
                nc.tensor.matmul(out=gp[0:1, :], lhsT=w["attw"][:], rhs=ef2[:],
                                 start=True, stop=True)
                gs = ep.tile([1, CH], f32, tag="gs")
                nc.scalar.activation(gs[:], gp[0:1, :], Act.Sigmoid)
                gbc = pp.tile([128, CH], f32, tag="ps")
                nc.tensor.matmul(out=gbc[:], lhsT=w["ones1"][:], rhs=gs[:],
                                 start=True, stop=True)
                ef = ep.tile([128, CH], f32, tag="ef")
                nc.vector.tensor_tensor(out=ef[:], in0=ef2[:], in1=gbc[:],
                                        op=Alu.mult)

                cm1p = pp.tile([128, CH], f32, tag="ps")
                nc.tensor.matmul(out=cm1p[:], lhsT=w["cw1"][:], rhs=ef[:],
                                 start=True, stop=True)
                cm1 = ep.tile([128, CH], f32, tag="cm1")
                nc.scalar.activation(cm1[:], cm1p[:], Act.Silu)
                cmp_ = pp.tile([128, CH], f32, tag="ps")
                nc.tensor.matmul(out=cmp_[0:14, :], lhsT=w["cw2"][:], rhs=cm1[:],
                                 start=True, stop=True)
                cm = ep.tile([14, CH], f32, tag="cm")
                nc.scalar.activation(cm[:], cmp_[0:14, :], Act.Copy)

                # ---- RollerPooling ----
                pooled_p = pp.tile([128, CH], f32, tag="ps")
                for half in range(2):
                    hs = slice(half * 98, (half + 1) * 98)
                    pall = pp.tile([128, CH], f32, tag="ps")
                    nc.tensor.matmul(out=pall[0:98, :], lhsT=w["pstk"][:, hs],
                                     rhs=cm[:], start=True, stop=True)
                    rmp = pp.tile([128, CH], f32, tag="ps")
                    nc.tensor.matmul(out=rmp[0:98, :], lhsT=w["repm"][:, hs],
                                     rhs=RAx[0:14, :], start=True, stop=True)
                    rms = ep.tile([98, CH], f32, tag=f"rms{half}")
                    nc.scalar.activation(rms[:], rmp[0:98, :], Act.Copy)
                    msk = ep.tile([98, CH], f32, tag=f"msk{half}")
                    nc.vector.tensor_tensor(out=msk[:], in0=pall[0:98, :],
                                            in1=rms[:], op=Alu.mult)
                    nc.tensor.matmul(out=pooled_p[0:14, :],
                                     lhsT=w[f"selm{half}"][:], rhs=msk[:],
                                     start=(half == 0), stop=(half == 1))
                pooled = ep.tile([14, CH], f32, tag="pooled")
                nc.scalar.activation(pooled[:], pooled_p[0:14, :], Act.Copy)

                # ---- trans + ef transpose for scatter ----
                for s in range(4):
                    ssl = slice(s * 128, (s + 1) * 128)
                    rr = rec_r[:, s, :]
                    rc = rec_c[:, s, :]
                    ptp = pp.tile([128, CH], f32, tag="ps")
                    nc.tensor.transpose(out=ptp[:, 0:14], in_=pooled[:, ssl],
                                        identity=ident[0:14, 0:14])
                    cd = sp.tile([128, 42], f32, tag="cd")
                    nc.vector.tensor_tensor(
                        out=apx(cd[:], [(3, 14), (1, 3)]),
                        in0=apx(rr[:, 0:42], [(3, 14), (1, 3)]),
                        in1=apx(rc[:, 266:269], [(0, 14), (1, 3)]),
                        op=Alu.subtract)
                    nc.vector.tensor_tensor(
                        out=apx(xsrc[:, s, 0:42], [(3, 14), (1, 3)]),
                        in0=apx(cd[:], [(3, 14), (1, 3)]),
                        in1=apx(ptp[:, 0:14], [(1, 14), (0, 3)]),
                        op=Alu.mult)
                    etp = pp.tile([128, CH], f32, tag="ps")
                    nc.tensor.transpose(out=etp[:, 0:128], in_=ef[:, ssl],
                                        identity=ident[:])
                    nc.scalar.activation(esrc[:, s, :], etp[:, 0:128], Act.Copy)

                # within-sub duplicate pre-sum (S-matmul) + first-occurrence
                # masking, then per-sub scatters (WAW-serialized -> exact adds)
                for s in range(4):
                    ci = cki * 4 + s
                    nv = max(0, min(128, nvalid - s * 128))
                    for fnm, mnm, pay, dst, es in (
                        ("colf", "isfc", esrc, agg_d, F),
                        ("rowf", "isfr", xsrc, xsum_d, 64),
                    ):
                        iv = sub[fnm][:, ci:ci + 1]
                        tpp = pp.tile([128, CH], f32, tag="ps")
                        nc.tensor.transpose(out=tpp[:, 0:128],
                                            in_=iv.to_broadcast([128, 128]),
                                            identity=ident[:])
                        Smat = sp.tile([128, 128], f32, tag=f"S_{fnm}")
                        nc.vector.tensor_tensor(
                            out=Smat[:], in0=iv.to_broadcast([128, 128]),
                            in1=tpp[:, 0:128], op=Alu.is_equal)
                        pps = pp.tile([128, CH], f32, tag="ps")
                        nc.tensor.matmul(out=pps[:, 0:es], lhsT=Smat[:],
                                         rhs=pay[:, s, :], start=True, stop=True)
                        nc.scalar.activation(pay[:, s, :], pps[:, 0:es],
                                             Act.Copy, scale=sub[mnm][:, ci:ci + 1])
                    sisl = slice(cki * 32 + s * 8, cki * 32 + (s + 1) * 8)
                    nc.gpsimd.dma_scatter_add(agg_d[:], esrc[:, s:s + 1, :],
                                              tab["col_s"][:, sisl], 128, nv, F)
                    nc.gpsimd.dma_scatter_add(xsum_d[:], xsrc[:, s:s + 1, :],
                                              tab["row_s"][:, sisl], 128, nv, 64)

            # ========================= collectives ===========================
            nc.gpsimd.collective_compute(
                "ReduceScatter", mybir.AluOpType.add, replica_groups=groups,
                ins=[agg_d[:]], outs=[aggrs_d[:]])
            nc.gpsimd.collective_compute(
                "ReduceScatter", mybir.AluOpType.add, replica_groups=groups,
                ins=[xsum_d[:]], outs=[xsrs_d[:]])

            # ========================= node phase ============================
            for t in range(NT):
                nb = min(128, NSH - t * 128)
                nsl = slice(t * 128, t * 128 + nb)
                hB = sp.tile([128, F], f32, tag="hB")
                ag = sp.tile([128, F], f32, tag="ag")
                ict = sp.tile([128, 1], f32, tag="ict")
                nc.sync.dma_start(out=hB[:nb, :], in_=hsh_d[nsl, :])
                nc.sync.dma_start(out=ag[:nb, :], in_=aggrs_d[nsl, :])
                nc.sync.dma_start(out=ict[:nb, :], in_=icc_d[nsl, :])
                agg = sp.tile([128, F], f32, tag="agg")
                nc.scalar.activation(agg[:], ag[:], Act.Copy, scale=ict[:])

                hAp = pp.tile([128, CH], f32, tag="ps")
                nc.tensor.transpose(out=hAp[:, 0:128], in_=hB[:], identity=ident[:])
                hA = sp.tile([128, F], f32, tag="hA")
                nc.scalar.activation(hA[:], hAp[:, 0:128], Act.Copy)
                aAp = pp.tile([128, CH], f32, tag="ps")
                nc.tensor.transpose(out=aAp[:, 0:128], in_=agg[:], identity=ident[:])
                aA = sp.tile([128, F], f32, tag="aA")
                nc.scalar.activation(aA[:], aAp[:, 0:128], Act.Copy)

                n1p = pp.tile([128, CH], f32, tag="ps")
                nc.tensor.matmul(out=n1p[:, 0:128], lhsT=w["nw1a"][:], rhs=hA[:],
                                 start=True, stop=False)
                nc.tensor.matmul(out=n1p[:, 0:128], lhsT=w["nw1b"][:], rhs=aA[:],
                                 start=False, stop=True)
                n1 = sp.tile([128, F], f32, tag="n1")
                nc.scalar.activation(n1[:], n1p[:, 0:128], Act.Silu)
                nhp = pp.tile([128, CH], f32, tag="ps")
                nc.tensor.matmul(out=nhp[:, 0:128], lhsT=w["nw2"][:], rhs=n1[:],
                                 start=True, stop=True)
                nhA = sp.tile([128, F], f32, tag="nhA")
                nc.scalar.activation(nhA[:], nhp[:, 0:128], Act.Copy)
                nhBp = pp.tile([128, CH], f32, tag="ps")
                nc.tensor.transpose(out=nhBp[:, 0:128], in_=nhA[:], identity=ident[:])

                hr = sp.tile([128, F], f32, tag="hr")
                nc.vector.tensor_tensor(out=hr[:], in0=hB[:], in1=nhBp[:, 0:128],
                                        op=Alu.add)
                scr = sp.tile([128, F], f32, tag="scr")
                mu = sp.tile([128, 1], f32, tag="mu")
                nc.scalar.activation(scr[:], hr[:], Act.Copy, accum_out=mu[:])
                nc.vector.tensor_scalar_mul(out=mu[:], in0=mu[:], scalar1=1.0 / F)
                xc = sp.tile([128, F], f32, tag="xc")
                nc.vector.tensor_tensor(out=xc[:], in0=hr[:],
                                        in1=apx(mu[:], [(0, F)]), op=Alu.subtract)
                v2 = sp.tile([128, 1], f32, tag="v2")
                nc.scalar.activation(scr[:], xc[:], Act.Square, accum_out=v2[:])
                nc.vector.tensor_scalar(out=v2[:], in0=v2[:], scalar1=1.0 / F,
                                        scalar2=LN_EPS, op0=Alu.mult, op1=Alu.add)
                rstd = sp.tile([128, 1], f32, tag="rstd")
                nc.scalar.activation(rstd[:], v2[:], Act.Sqrt)
                nc.vector.reciprocal(rstd[:], rstd[:])
                hnew = sp.tile([128, F], f32, tag="hnew")
                nc.scalar.activation(hnew[:], xc[:], Act.Copy, scale=rstd[:])
                nc.sync.dma_start(out=hout_d[nsl, :], in_=hnew[:nb, :])

                xB = sp.tile([128, 42], f32, tag="xB")
                xs = sp.tile([128, 64], f32, tag="xs")
                irt = sp.tile([128, 1], f32, tag="irt")
                nc.sync.dma_start(out=xB[:nb, :], in_=xsh_d[nsl, :])
                nc.sync.dma_start(out=xs[:nb, :], in_=xsrs_d[nsl, :])
                nc.sync.dma_start(out=irt[:nb, :], in_=icr_d[nsl, :])
                xd = sp.tile([128, 42], f32, tag="xd")
                nc.scalar.activation(xd[:], xs[:, 0:42], Act.Copy, scale=irt[:])
                xn = sp.tile([128, 42], f32, tag="xn")
                nc.vector.tensor_tensor(out=xn[:], in0=xB[:], in1=xd[:],
                                        op=Alu.add)
                nc.sync.dma_start(out=xout_d[nsl, :], in_=xn[:nb, :])

    nc.compile()
    return nc


_NC_CACHE = None
LAST_RESULTS = None


def kernel(**inputs):
    global _NC_CACHE
    rec, h_bf, tabs, W, shards = _host_prep(inputs)
    if _NC_CACHE is None:
        _NC_CACHE = _build_bass()
    nc = _NC_CACHE

    in_maps = []
    for k in range(NCORES):
        m = dict(rec=rec, h_bf=h_bf,
                 row_g=tabs[k][0], col_g=tabs[k][1],
                 row_s=tabs[k][2], col_s=tabs[k][3],
                 rowf=tabs[k][4], colf=tabs[k][5],
                 isfr=tabs[k][6], isfc=tabs[k][7])
        m.update(W)
        m.update(shards[k])
        in_maps.append(m)

    from concourse.bass_utils import run_bass_kernel_spmd
    trace = os.environ.get("BASS_KERNEL_TRACE") == "1"
    res = run_bass_kernel_spmd(nc, in_maps, list(range(NCORES)), trace=trace)
    global LAST_RESULTS
    LAST_RESULTS = res
    h_new = np.concatenate(
        [res.results[k]["h_out"][:min(NSH, N - k * NSH)] for k in range(NCORES)], 0)
    x_new = np.concatenate(
        [res.results[k]["x_out"][:min(NSH, N - k * NSH)] for k in range(NCORES)], 0)
    return h_new.astype(np.float32), x_new.reshape(N, C, D).astype(np.float32)
